# revision 23
# baseline (speedup 1.0000x reference)
"""Trainium2 Bass kernel for nn_MultiHeadAttention (B=4, T=2048, D=1024,
H=16, d_k=64) on 8 NeuronCores.

Sharding: tensor-parallel over heads - core c computes heads {2c, 2c+1} for
ALL batches (W_q/W_k/W_v column-sharded, W_o row-sharded). The final
all-reduce of the output projection is replaced by a host-side sum of the 8
partial outputs. Per-batch attention length (ceil(valid_len/128) Tk tiles)
is baked into the single SPMD program; every core owns 2 heads of every
batch so the instruction stream is identical and load-balanced.

Device tricks (v2):
  - scores^T layout (Tk on partitions, Tq on free). The two heads' QK^T
    matmuls write one [128, 2, 512] PSUM tile (adjacent row-tiled K=64
    matmuls -> concurrent on the PE array), and ONE Exp activation
    normalizes instruction overhead over both heads; the padding mask rides
    the per-partition bias operand.
  - softmax denominator: a ones-column folded into V (lhsT = [V_h | 1]).
  - b_v is folded into b_o on the host (softmax rows sum to 1).
  - Unnormalized [65, 512] outputs are staged to SBUF by DVE; den rows are
    gathered by SBUF-SBUF DMA so the reciprocal runs batched [8, 512];
    1/den is broadcast for BOTH heads with a single K=2 bf16 matmul
    (selector lhsT I2), then DVE multiplies into AO.
  - V projection accumulates 4 Tk tiles in one PSUM bank -> one [128, 512]
    stage-out copy per 512-row window.
  - K/V projections and DMAs only cover ceil(valid_len/512) windows.
  - trn2 encodes at most one semaphore wait per instruction; a post-pass
    splits multi-wait instructions Tile emits into single-wait events.
"""
import os
import sys

for _p in ("/opt/trn_rl_repo", "/root/.axon_site/_ro/trn_rl_repo"):
    if os.path.isdir(_p) and _p not in sys.path:
        sys.path.append(_p)

import numpy as np
import ml_dtypes

import concourse.bass as bass
import concourse.mybir as mybir
import concourse.tile as tile
from concourse.bass import ts
from concourse.bass_utils import run_bass_kernel_spmd

D = 1024
T = 2048
H = 16
DK = 64
P = 128
KC = D // P          # 8 contraction chunks for the projections
NT = T // 512        # 4 Tq chunks of 512
TC = T // P          # 16 Tk tiles / T chunks
NCORES = 8
CPB = (H // NCORES) * DK   # projection cols per core = 128 (2 heads)
MASK_NEG = -30000.0
EXP_SCALE = 0.125

F32 = mybir.dt.float32
BF16 = mybir.dt.bfloat16
AF = mybir.ActivationFunctionType
BF16_NP = ml_dtypes.bfloat16


def _split_multi_waits(nc):
    """trn2 instructions encode at most one sync wait; split the rest into
    standalone single-wait event-semaphore ops."""
    n_split = 0
    for f in nc.m.functions:
        for blk in f.blocks:
            insts = blk.instructions
            out = []
            changed = False
            for inst in insts:
                si = inst.sync_info
                if si is not None and len(si.on_wait) > 1:
                    waits = list(si.on_wait)
                    for k, wt in enumerate(waits[:-1]):
                        ev = mybir.InstEventSemaphore(
                            name=f"{inst.name}_wsplit{k}",
                            engine=inst.engine,
                            ins=[],
                            outs=[],
                            bass_nofuse=True,
                            sync_info=mybir.SyncInfo(on_wait=[wt], on_update=[]),
                        )
                        out.append(ev)
                        n_split += 1
                    inst.sync_info = mybir.SyncInfo(
                        on_wait=[waits[-1]], on_update=si.on_update
                    )
                    changed = True
                out.append(inst)
            if changed:
                blk.instructions = out
    return n_split


def build_nc(NB, J_list, NW_list):
    """Build the SPMD program.

    NB      : number of batch slots handled per core
    J_list  : per batch slot, number of 128-row Tk tiles of attention
    NW_list : per batch slot, number of 512-row K/V windows (= ceil(J/4))
    """
    nc = bass.Bass()

    xq_d = [nc.declare_dram_parameter(f"xq{s}", [NT, P, KC, 512], BF16,
                                      isOutput=False) for s in range(NB)]
    xk_d = [nc.declare_dram_parameter(f"xk{s}", [NW_list[s], P, KC, 512], BF16,
                                      isOutput=False) for s in range(NB)]
    xv_d = [nc.declare_dram_parameter(f"xv{s}", [NW_list[s], P, KC, 512], BF16,
                                      isOutput=False) for s in range(NB)]
    wq_d = nc.declare_dram_parameter("wq", [P, KC, CPB], BF16, isOutput=False)
    wk_d = nc.declare_dram_parameter("wk", [P, KC, CPB], BF16, isOutput=False)
    wv_d = nc.declare_dram_parameter("wv", [P, KC, CPB], BF16, isOutput=False)
    wo_d = nc.declare_dram_parameter("wo", [P, D], BF16, isOutput=False)
    bq_d = nc.declare_dram_parameter("bq", [P, 1], F32, isOutput=False)
    bk_d = nc.declare_dram_parameter("bk", [P, 1], F32, isOutput=False)
    mb_d = [nc.declare_dram_parameter(f"mb{s}", [P, TC], F32, isOutput=False)
            for s in range(NB)]
    i2_d = nc.declare_dram_parameter("i2", [2, P], BF16, isOutput=False)
    o_d = [nc.declare_dram_parameter(f"o{s}", [T, D], BF16, isOutput=True)
           for s in range(NB)]

    with tile.TileContext(nc) as tc:
        with (
            tc.tile_pool(name="pers", bufs=1) as pers,
            tc.tile_pool(name="stream", bufs=2) as stream,
            tc.tile_pool(name="attn", bufs=3) as attn_pool,
            tc.tile_pool(name="small", bufs=3) as small,
            tc.tile_pool(name="outp", bufs=4) as outp,
            tc.tile_pool(name="ps_qk", bufs=2, space="PSUM") as ps_qk,
            tc.tile_pool(name="ps_pv", bufs=2, space="PSUM") as ps_pv,
            tc.tile_pool(name="ps_gen", bufs=2, space="PSUM") as ps_gen,
        ):
            # ---- persistent tensors -------------------------------------
            # big DMAs are split into 4 partition-chunks: dma_start calls
            # round-robin over the 16 DMA queues, so a split transfer moves
            # 4x faster than one 128-descriptor transfer on a single queue
            def dma4(dst, src):
                for p4 in range(4):
                    nc.sync.dma_start(dst[ts(p4, 32)], src[ts(p4, 32)])

            wq = pers.tile([P, KC, CPB], BF16, name="wq")
            wk = pers.tile([P, KC, CPB], BF16, name="wk")
            wv = pers.tile([P, KC, CPB], BF16, name="wv")
            wo = pers.tile([P, D], BF16, name="wo")
            bq = pers.tile([P, 1], F32, name="bq")
            bk = pers.tile([P, 1], F32, name="bk")
            dma4(wq, wq_d)
            dma4(wk, wk_d)
            dma4(wv, wv_d)
            dma4(wo, wo_d)
            nc.sync.dma_start(bq[:], bq_d[:])
            nc.sync.dma_start(bk[:], bk_d[:])
            mb = []
            for s in range(NB):
                t = pers.tile([P, TC], F32, name=f"mb{s}")
                nc.sync.dma_start(t[:], mb_d[s][:])
                mb.append(t)

            # bcast selector: out p<64 <- rec_h0, p>=64 <- rec_h1
            I2 = pers.tile([2, P], BF16, name="I2")
            nc.sync.dma_start(I2[:], i2_d[:])

            # pull the one-time Exp table load off the first attention unit
            warm = pers.tile([P, 1], F32, name="warm")
            nc.vector.memset(warm[:], 0.0)
            nc.scalar.activation(warm[:], warm[:], AF.Exp)

            QT = [pers.tile([P, T], BF16, name=f"QT{s}") for s in range(NB)]
            KT = [pers.tile([P, T], BF16, name=f"KT{s}") for s in range(NB)]
            # V with a ones column folded in at free index 64 of each head
            V = [pers.tile([P, TC, 2, DK + 1], BF16, name=f"V{s}")
                 for s in range(NB)]
            for s in range(NB):
                nc.vector.memset(V[s][:, :, :, DK], 1.0)

            AO = [pers.tile([P, T], BF16, name=f"AO{s}") for s in range(NB)]
            NR = 2 * NT  # (tq, head) rows per slot
            uo = [pers.tile([DK + 1, NR, 512], BF16, name=f"uo{s}")
                  for s in range(NB)]
            dens = [pers.tile([NR, 512], BF16, name=f"dens{s}")
                    for s in range(NB)]
            recs = [pers.tile([NR, 512], BF16, name=f"rec{s}")
                    for s in range(NB)]

            def proj_q(s):
                for n in range(NT):
                    xq_w = stream.tile([P, KC, 512], BF16, tag="xq_w")
                    dma4(xq_w, xq_d[s][n])
                    ps_q = ps_gen.tile([P, 512], F32, tag="gen")
                    for kc in range(KC):
                        nc.tensor.matmul(ps_q[:], wq[:, kc, :],
                                         xq_w[:, kc, :],
                                         start=(kc == 0), stop=(kc == KC - 1))
                    nc.vector.tensor_scalar_add(QT[s][:, ts(n, 512)],
                                                ps_q[:], bq[:, 0:1])

            def proj_k(s):
                for n in range(NW_list[s]):
                    xk_w = stream.tile([P, KC, 512], BF16, tag="xk_w")
                    dma4(xk_w, xk_d[s][n])
                    ps_k = ps_gen.tile([P, 512], F32, tag="gen")
                    for kc in range(KC):
                        nc.tensor.matmul(ps_k[:], wk[:, kc, :],
                                         xk_w[:, kc, :],
                                         start=(kc == 0), stop=(kc == KC - 1))
                    nc.vector.tensor_scalar_add(KT[s][:, ts(n, 512)],
                                                ps_k[:], bk[:, 0:1])

            def proj_v(s):
                # V projection: 4 Tk tiles share one PSUM bank per window
                for n in range(NW_list[s]):
                    xv_w = stream.tile([P, KC, 512], BF16, tag="xv_w")
                    dma4(xv_w, xv_d[s][n])
                    ps_v = ps_gen.tile([P, 512], F32, tag="gen")
                    ps_v4 = ps_v[:].rearrange("p (t c) -> p t c", t=4)
                    for tc_i in range(4):
                        for kc in range(KC):
                            nc.tensor.matmul(ps_v4[:, tc_i, :],
                                             xv_w[:, kc, ts(tc_i, P)],
                                             wv[:, kc, :],
                                             start=(kc == 0),
                                             stop=(kc == KC - 1))
                    nc.vector.tensor_copy(
                        V[s][:, 4 * n:4 * n + 4, :, 0:DK],
                        ps_v4.rearrange("p t (h d) -> p t h d", d=DK))

            def proj_phase(s):
                proj_q(s)
                proj_k(s)
                proj_v(s)

            def attn_block(s, tq):
                J = J_list[s]
                if True:
                    po = [ps_pv.tile([DK + 1, 512], F32, tag="pv",
                                     name=f"po{h}") for h in range(2)]
                    for j in range(J):
                        ps_s = ps_qk.tile([P, 2, 512], F32, tag="qk")
                        nc.tensor.matmul(ps_s[:, 0, :],
                                         KT[s][0:DK, ts(j, P)],
                                         QT[s][0:DK, ts(tq, 512)],
                                         start=True, stop=True,
                                         tile_position=(0, 0))
                        nc.tensor.matmul(ps_s[:, 1, :],
                                         KT[s][DK:P, ts(j, P)],
                                         QT[s][DK:P, ts(tq, 512)],
                                         start=True, stop=True,
                                         tile_position=(DK, 0))
                        at = attn_pool.tile([P, 2, 512], BF16, tag="at")
                        nc.scalar.activation(at[:], ps_s[:], AF.Exp,
                                             bias=mb[s][:, j:j + 1],
                                             scale=EXP_SCALE)
                        for h01 in range(2):
                            nc.tensor.matmul(po[h01][:],
                                             V[s][:, j, h01, :],
                                             at[:, h01, :],
                                             start=(j == 0),
                                             stop=(j == J - 1))
                    for h01 in range(2):
                        r = 2 * tq + h01
                        nc.vector.tensor_copy(uo[s][:, r, :], po[h01][:])
                        nc.sync.dma_start(dens[s][r:r + 1, :],
                                          uo[s][DK:DK + 1, r, :])

            def norm_outproj_phase(s, last):
                with nc.allow_low_precision(reason="bf16 softmax denom"):
                    nc.vector.reciprocal(recs[s][:], dens[s][:])
                for tq in range(NT):
                    rst2 = small.tile([2, 512], BF16, tag="rst")
                    nc.sync.dma_start(rst2[:], recs[s][2 * tq:2 * tq + 2, :])
                    ps_b = ps_gen.tile([P, 512], F32, tag="gen")
                    nc.tensor.matmul(ps_b[:], I2[:], rst2[:],
                                     start=True, stop=True)
                    for h01 in range(2):
                        nc.vector.tensor_mul(
                            out=AO[s][ts(h01, DK), ts(tq, 512)],
                            in0=ps_b[ts(h01, DK), :],
                            in1=uo[s][0:DK, 2 * tq + h01, :])

                # ---- output projection (partial; host sums cores) -------
                for m in range(TC):
                    ot = outp.tile([P, D], BF16, tag="ot")
                    for n2 in range(2):
                        ps_op = ps_gen.tile([P, 512], F32, tag="gen")
                        nc.tensor.matmul(ps_op[:], AO[s][:, ts(m, P)],
                                         wo[:, ts(n2, 512)],
                                         start=True, stop=True)
                        if last or (2 * m + n2) % 3 == 0:
                            nc.scalar.activation(ot[:, ts(n2, 512)], ps_op[:],
                                                 AF.Identity)
                        else:
                            nc.vector.tensor_copy(ot[:, ts(n2, 512)], ps_op[:])
                    for p4 in range(4):
                        nc.sync.dma_start(o_d[s][m * P + p4 * 32:
                                                 m * P + p4 * 32 + 32, :],
                                          ot[ts(p4, 32)])

            # Software-pipelined emission. Slots are ordered ascending by J.
            # Strict-priority scheduling means work emitted EARLIER preempts
            # later work, so each proj/norm sub-phase is emitted right AFTER
            # the attention blocks whose ACT-paced stretch should hide it,
            # and right BEFORE the blocks that depend on it. The big slot's
            # projections trickle in piecewise (q/k/v) so no single burst
            # stalls the Exp stream.
            if NB == 4:
                big = NB - 1
                proj_phase(0)
                for tq in range(NT):
                    attn_block(0, tq)
                proj_phase(1)
                proj_q(big)
                for tq in range(NT):
                    attn_block(1, tq)
                proj_phase(2)
                proj_k(big)
                for tq in range(NT):
                    attn_block(2, tq)
                proj_v(big)
                norm_outproj_phase(0, last=False)
                attn_block(big, 0)
                norm_outproj_phase(1, last=False)
                attn_block(big, 1)
                norm_outproj_phase(2, last=False)
                attn_block(big, 2)
                attn_block(big, 3)
                norm_outproj_phase(big, last=True)
            else:
                proj_phase(0)
                if NB > 1:
                    proj_phase(1)
                for s in range(NB):
                    for tq in range(NT):
                        attn_block(s, tq)
                    if s + 2 < NB:
                        proj_phase(s + 2)
                    if s >= 1:
                        norm_outproj_phase(s - 1, last=False)
                norm_outproj_phase(NB - 1, last=True)

    _split_multi_waits(nc)
    return nc


_CACHE = {}


def _get_nc(NB, J_list, NW_list):
    key = (NB, tuple(J_list), tuple(NW_list))
    if key not in _CACHE:
        _CACHE[key] = build_nc(NB, list(J_list), list(NW_list))
    return _CACHE[key]


def _xt(x, nw):
    """[T, D] -> [nw, P, KC, 512] transposed window-major layout."""
    xt = x.T.reshape(KC, P, NT, 512).transpose(2, 1, 0, 3)[:nw]
    return np.ascontiguousarray(xt).astype(BF16_NP)


def kernel(**inputs):
    query = np.asarray(inputs["query"], dtype=np.float32)
    key = np.asarray(inputs["key"], dtype=np.float32)
    value = np.asarray(inputs["value"], dtype=np.float32)
    vl = np.asarray(inputs["valid_length"]).astype(np.int64)
    W_q = np.asarray(inputs["W_q"], dtype=np.float32)
    b_q = np.asarray(inputs["b_q"], dtype=np.float32)
    W_k = np.asarray(inputs["W_k"], dtype=np.float32)
    b_k = np.asarray(inputs["b_k"], dtype=np.float32)
    W_v = np.asarray(inputs["W_v"], dtype=np.float32)
    b_v = np.asarray(inputs["b_v"], dtype=np.float32)
    W_o = np.asarray(inputs["W_o"], dtype=np.float32)
    b_o = np.asarray(inputs["b_o"], dtype=np.float32)

    B = query.shape[0]
    NB = B

    # slot s handles batch order[s] (ascending valid_len so the largest
    # attention phase runs last, hiding earlier slots' epilogues)
    order = np.argsort(vl, kind="stable")
    J_list, NW_list = [], []
    for s in range(NB):
        v = int(vl[order[s]])
        J = TC if v == 0 else max(1, -(-v // P))
        J_list.append(J)
        NW_list.append(-(-J // 4))

    nc = _get_nc(NB, J_list, NW_list)

    # host-side shard prep
    xq_np, xk_np, xv_np, mb_np = [], [], [], []
    for s in range(NB):
        b = int(order[s])
        v = int(vl[b])
        q_b = query[b] if v != 0 else np.zeros_like(query[b])
        xq_np.append(_xt(q_b, NT))
        xk_np.append(_xt(key[b], NW_list[s]))
        xv_np.append(_xt(value[b], NW_list[s]))
        rows = np.arange(P)[:, None] + P * np.arange(TC)[None, :]
        if v == 0:
            m = np.zeros((P, TC), np.float32)
        else:
            m = np.where(rows < v, 0.0, MASK_NEG).astype(np.float32)
        mb_np.append(m)

    # b_v folds into b_o: softmax rows sum to 1, so attn_out = P@(xWv) + b_v
    b_o_eff = b_o + b_v @ W_o

    i2_np = np.zeros((2, P), BF16_NP)
    i2_np[0, 0:DK] = 1
    i2_np[1, DK:P] = 1

    in_maps = []
    for c in range(NCORES):
        c0 = c * CPB
        cols = slice(c0, c0 + CPB)
        im = {
            "wq": np.ascontiguousarray(
                W_q.reshape(KC, P, H * DK).transpose(1, 0, 2)[:, :, cols]
            ).astype(BF16_NP),
            "wk": np.ascontiguousarray(
                W_k.reshape(KC, P, H * DK).transpose(1, 0, 2)[:, :, cols]
            ).astype(BF16_NP),
            "wv": np.ascontiguousarray(
                W_v.reshape(KC, P, H * DK).transpose(1, 0, 2)[:, :, cols]
            ).astype(BF16_NP),
            "wo": np.ascontiguousarray(W_o[cols]).astype(BF16_NP),
            "bq": b_q[cols].reshape(P, 1).astype(np.float32),
            "bk": b_k[cols].reshape(P, 1).astype(np.float32),
            "i2": i2_np,
        }
        for s in range(NB):
            im[f"xq{s}"] = xq_np[s]
            im[f"xk{s}"] = xk_np[s]
            im[f"xv{s}"] = xv_np[s]
            im[f"mb{s}"] = mb_np[s]
        in_maps.append(im)

    res = run_bass_kernel_spmd(nc, in_maps, list(range(NCORES)))

    out = np.zeros((B, T, D), np.float32)
    for s in range(NB):
        b = int(order[s])
        acc = np.zeros((T, D), np.float32)
        for c in range(NCORES):
            acc += np.asarray(res.results[c][f"o{s}"]).astype(np.float32)
        out[b] = acc + b_o_eff[None, :]
    return out


# revision 36
# speedup vs baseline: 1.5714x; 1.5714x over previous
"""Trainium2 Bass kernel for nn_MultiHeadAttention (B=4, T=2048, D=1024,
H=16, d_k=64) on 8 NeuronCores.

Sharding: tensor-parallel over heads - core c computes heads {2c, 2c+1} for
ALL batches (W_q/W_k/W_v column-sharded, W_o row-sharded). The final
all-reduce of the output projection is replaced by a host-side sum of the 8
partial outputs. Per-batch attention length (ceil(valid_len/128) Tk tiles)
is baked into the single SPMD program; every core owns 2 heads of every
batch so the instruction stream is identical and load-balanced.

Device tricks (v2):
  - scores^T layout (Tk on partitions, Tq on free). The two heads' QK^T
    matmuls write one [128, 2, 512] PSUM tile (adjacent row-tiled K=64
    matmuls -> concurrent on the PE array), and ONE Exp activation
    normalizes instruction overhead over both heads; the padding mask rides
    the per-partition bias operand.
  - softmax denominator: a ones-column folded into V (lhsT = [V_h | 1]).
  - b_v is folded into b_o on the host (softmax rows sum to 1).
  - Unnormalized [65, 512] outputs are staged to SBUF by DVE; den rows are
    gathered by SBUF-SBUF DMA so the reciprocal runs batched [8, 512];
    1/den is broadcast for BOTH heads with a single K=2 bf16 matmul
    (selector lhsT I2), then DVE multiplies into AO.
  - V projection accumulates 4 Tk tiles in one PSUM bank -> one [128, 512]
    stage-out copy per 512-row window.
  - K/V projections and DMAs only cover ceil(valid_len/512) windows.
  - trn2 encodes at most one semaphore wait per instruction; a post-pass
    splits multi-wait instructions Tile emits into single-wait events.
"""
import os
import sys

for _p in ("/opt/trn_rl_repo", "/root/.axon_site/_ro/trn_rl_repo"):
    if os.path.isdir(_p) and _p not in sys.path:
        sys.path.append(_p)

import numpy as np
import ml_dtypes

import concourse.bass as bass
import concourse.mybir as mybir
import concourse.tile as tile
from concourse.bass import ts
from concourse.bass_utils import run_bass_kernel_spmd

D = 1024
T = 2048
H = 16
DK = 64
P = 128
KC = D // P          # 8 contraction chunks for the projections
NT = T // 512        # 4 Tq chunks of 512
TC = T // P          # 16 Tk tiles / T chunks
NCORES = 8
CPB = (H // NCORES) * DK   # projection cols per core = 128 (2 heads)
MASK_NEG = -30000.0
EXP_SCALE = 0.125

F32 = mybir.dt.float32
BF16 = mybir.dt.bfloat16
AF = mybir.ActivationFunctionType
BF16_NP = ml_dtypes.bfloat16


def _split_multi_waits(nc):
    """trn2 instructions encode at most one sync wait; split the rest into
    standalone single-wait event-semaphore ops."""
    n_split = 0
    for f in nc.m.functions:
        for blk in f.blocks:
            insts = blk.instructions
            out = []
            changed = False
            for inst in insts:
                si = inst.sync_info
                if si is not None and len(si.on_wait) > 1:
                    waits = list(si.on_wait)
                    for k, wt in enumerate(waits[:-1]):
                        ev = mybir.InstEventSemaphore(
                            name=f"{inst.name}_wsplit{k}",
                            engine=inst.engine,
                            ins=[],
                            outs=[],
                            bass_nofuse=True,
                            sync_info=mybir.SyncInfo(on_wait=[wt], on_update=[]),
                        )
                        out.append(ev)
                        n_split += 1
                    inst.sync_info = mybir.SyncInfo(
                        on_wait=[waits[-1]], on_update=si.on_update
                    )
                    changed = True
                out.append(inst)
            if changed:
                blk.instructions = out
    return n_split


def build_nc(NB, J_list, NW_list):
    """Build the SPMD program.

    NB      : number of batch slots handled per core
    J_list  : per batch slot, number of 128-row Tk tiles of attention
    NW_list : per batch slot, number of 512-row K/V windows (= ceil(J/4))
    """
    nc = bass.Bass()

    xq_d = [nc.declare_dram_parameter(f"xq{s}", [NT, P, KC, 512], BF16,
                                      isOutput=False) for s in range(NB)]
    xk_d = [nc.declare_dram_parameter(f"xk{s}", [NW_list[s], P, KC, 512], BF16,
                                      isOutput=False) for s in range(NB)]
    xv_d = [nc.declare_dram_parameter(f"xv{s}", [NW_list[s], P, KC, 512], BF16,
                                      isOutput=False) for s in range(NB)]
    wq_d = nc.declare_dram_parameter("wq", [P, KC, CPB], BF16, isOutput=False)
    wk_d = nc.declare_dram_parameter("wk", [P, KC, CPB], BF16, isOutput=False)
    wv_d = nc.declare_dram_parameter("wv", [P, KC, CPB], BF16, isOutput=False)
    wo_d = nc.declare_dram_parameter("wo", [P, D], BF16, isOutput=False)
    bq_d = nc.declare_dram_parameter("bq", [P, 1], F32, isOutput=False)
    bk_d = nc.declare_dram_parameter("bk", [P, 1], F32, isOutput=False)
    mb_d = [nc.declare_dram_parameter(f"mb{s}", [P, TC], F32, isOutput=False)
            for s in range(NB)]
    i2_d = nc.declare_dram_parameter("i2", [2, P], BF16, isOutput=False)
    o_d = [nc.declare_dram_parameter(f"o{s}", [T, D], BF16, isOutput=True)
           for s in range(NB)]

    with tile.TileContext(nc) as tc:
        with (
            tc.tile_pool(name="pers", bufs=1) as pers,
            tc.tile_pool(name="stream", bufs=3) as stream,
            tc.tile_pool(name="attn", bufs=3) as attn_pool,
            tc.tile_pool(name="small", bufs=3) as small,
            tc.tile_pool(name="outp", bufs=4) as outp,
            tc.tile_pool(name="ps_qk", bufs=2, space="PSUM") as ps_qk,
            tc.tile_pool(name="ps_pv", bufs=2, space="PSUM") as ps_pv,
            tc.tile_pool(name="ps_gen", bufs=2, space="PSUM") as ps_gen,
        ):
            # ---- persistent tensors -------------------------------------
            # The Sync engine executes dma_start kicks IN ORDER; only
            # dependency-free prefetch loads may go through it. DMAs whose
            # source awaits compute (den gathers, rst2, o stores) go through
            # the otherwise-idle GpSimd queue so they never stall prefetch.
            wq = pers.tile([P, KC, CPB], BF16, name="wq")
            wk = pers.tile([P, KC, CPB], BF16, name="wk")
            wv = pers.tile([P, KC, CPB], BF16, name="wv")
            wo = pers.tile([P, D], BF16, name="wo")
            bq = pers.tile([P, 1], F32, name="bq")
            bk = pers.tile([P, 1], F32, name="bk")
            nc.sync.dma_start(wq[:], wq_d[:])

            def late_loads():
                nc.sync.dma_start(wk[:], wk_d[:])
                nc.sync.dma_start(wv[:], wv_d[:])
                nc.sync.dma_start(wo[:], wo_d[:])
                nc.sync.dma_start(bq[:], bq_d[:])
                nc.sync.dma_start(bk[:], bk_d[:])
                for s in range(NB):
                    nc.sync.dma_start(mb[s][:], mb_d[s][:])
                nc.sync.dma_start(I2[:], i2_d[:])
                # pull the one-time Exp table load off the first attention
                nc.vector.memset(warm[:], 0.0)
                nc.scalar.activation(warm[:], warm[:], AF.Exp)

            mb = [pers.tile([P, TC], F32, name=f"mb{s}") for s in range(NB)]
            I2 = pers.tile([2, P], BF16, name="I2")
            warm = pers.tile([P, 1], F32, name="warm")

            QT = [pers.tile([P, T], BF16, name=f"QT{s}") for s in range(NB)]
            KT = [pers.tile([P, T], BF16, name=f"KT{s}") for s in range(NB)]
            # V with a ones column folded in at free index 64 of each head
            V = [pers.tile([P, TC, 2, DK + 1], BF16, name=f"V{s}")
                 for s in range(NB)]
            for s in range(NB):
                nc.vector.memset(V[s][:, :, :, DK], 1.0)

            AO = [pers.tile([P, T], BF16, name=f"AO{s}") for s in range(NB)]
            NR = 2 * NT  # (tq, head) rows per slot
            uo = [pers.tile([DK + 1, NR, 512], BF16, name=f"uo{s}")
                  for s in range(NB)]
            dens = [pers.tile([NR, 512], BF16, name=f"dens{s}")
                    for s in range(NB)]
            recs = [pers.tile([NR, 512], BF16, name=f"rec{s}")
                    for s in range(NB)]

            def proj_q(s):
                for n in range(NT):
                    xq_w = stream.tile([P, KC, 512], BF16, tag="xq_w")
                    nc.sync.dma_start(xq_w[:], xq_d[s][n])
                    ps_q = ps_gen.tile([P, 512], F32, tag="gen")
                    for kc in range(KC):
                        nc.tensor.matmul(ps_q[:], wq[:, kc, :],
                                         xq_w[:, kc, :],
                                         start=(kc == 0), stop=(kc == KC - 1))
                    nc.vector.tensor_scalar_add(QT[s][:, ts(n, 512)],
                                                ps_q[:], bq[:, 0:1])

            def proj_k(s):
                for n in range(NW_list[s]):
                    xk_w = stream.tile([P, KC, 512], BF16, tag="xk_w")
                    nc.sync.dma_start(xk_w[:], xk_d[s][n])
                    ps_k = ps_gen.tile([P, 512], F32, tag="gen")
                    for kc in range(KC):
                        nc.tensor.matmul(ps_k[:], wk[:, kc, :],
                                         xk_w[:, kc, :],
                                         start=(kc == 0), stop=(kc == KC - 1))
                    nc.vector.tensor_scalar_add(KT[s][:, ts(n, 512)],
                                                ps_k[:], bk[:, 0:1])

            def proj_v(s):
                # V projection: 4 Tk tiles share one PSUM bank per window
                for n in range(NW_list[s]):
                    xv_w = stream.tile([P, KC, 512], BF16, tag="xv_w")
                    nc.sync.dma_start(xv_w[:], xv_d[s][n])
                    ps_v = ps_gen.tile([P, 512], F32, tag="gen")
                    ps_v4 = ps_v[:].rearrange("p (t c) -> p t c", t=4)
                    for tc_i in range(4):
                        for kc in range(KC):
                            nc.tensor.matmul(ps_v4[:, tc_i, :],
                                             xv_w[:, kc, ts(tc_i, P)],
                                             wv[:, kc, :],
                                             start=(kc == 0),
                                             stop=(kc == KC - 1))
                    nc.vector.tensor_copy(
                        V[s][:, 4 * n:4 * n + 4, :, 0:DK],
                        ps_v4.rearrange("p t (h d) -> p t h d", d=DK))

            def proj_phase(s):
                proj_q(s)
                proj_k(s)
                proj_v(s)

            def attn_block(s, tq):
                J = J_list[s]
                if True:
                    po = [ps_pv.tile([DK + 1, 512], F32, tag="pv",
                                     name=f"po{h}") for h in range(2)]
                    for j in range(J):
                        ps_s = ps_qk.tile([P, 2, 512], F32, tag="qk")
                        nc.tensor.matmul(ps_s[:, 0, :],
                                         KT[s][0:DK, ts(j, P)],
                                         QT[s][0:DK, ts(tq, 512)],
                                         start=True, stop=True,
                                         tile_position=(0, 0))
                        nc.tensor.matmul(ps_s[:, 1, :],
                                         KT[s][DK:P, ts(j, P)],
                                         QT[s][DK:P, ts(tq, 512)],
                                         start=True, stop=True,
                                         tile_position=(DK, 0))
                        at = attn_pool.tile([P, 2, 512], BF16, tag="at")
                        nc.scalar.activation(at[:], ps_s[:], AF.Exp,
                                             bias=mb[s][:, j:j + 1],
                                             scale=EXP_SCALE)
                        for h01 in range(2):
                            nc.tensor.matmul(po[h01][:],
                                             V[s][:, j, h01, :],
                                             at[:, h01, :],
                                             start=(j == 0),
                                             stop=(j == J - 1))
                    for h01 in range(2):
                        r = 2 * tq + h01
                        nc.vector.tensor_copy(uo[s][:, r, :], po[h01][:])
                        nc.gpsimd.dma_start(dens[s][r:r + 1, :],
                                            uo[s][DK:DK + 1, r, :])

            def norm_outproj_phase(s, last):
                with nc.allow_low_precision(reason="bf16 softmax denom"):
                    nc.vector.reciprocal(recs[s][:], dens[s][:])
                for tq in range(NT):
                    rst2 = small.tile([2, 512], BF16, tag="rst")
                    nc.gpsimd.dma_start(rst2[:], recs[s][2 * tq:2 * tq + 2, :])
                    ps_b = ps_gen.tile([P, 512], F32, tag="gen")
                    nc.tensor.matmul(ps_b[:], I2[:], rst2[:],
                                     start=True, stop=True)
                    for h01 in range(2):
                        nc.vector.tensor_mul(
                            out=AO[s][ts(h01, DK), ts(tq, 512)],
                            in0=ps_b[ts(h01, DK), :],
                            in1=uo[s][0:DK, 2 * tq + h01, :])

                # ---- output projection (partial; host sums cores) -------
                for m in range(TC):
                    ot = outp.tile([P, D], BF16, tag="ot")
                    for n2 in range(2):
                        ps_op = ps_gen.tile([P, 512], F32, tag="gen")
                        nc.tensor.matmul(ps_op[:], AO[s][:, ts(m, P)],
                                         wo[:, ts(n2, 512)],
                                         start=True, stop=True)
                        if last or (2 * m + n2) % 3 == 0:
                            nc.scalar.activation(ot[:, ts(n2, 512)], ps_op[:],
                                                 AF.Identity)
                        else:
                            nc.vector.tensor_copy(ot[:, ts(n2, 512)], ps_op[:])
                    nc.gpsimd.dma_start(o_d[s][ts(m, P), :], ot[:])

            # Software-pipelined emission. Slots are ordered ascending by J.
            # Strict-priority scheduling means work emitted EARLIER preempts
            # later work, so each proj/norm sub-phase is emitted right AFTER
            # the attention blocks whose ACT-paced stretch should hide it,
            # and right BEFORE the blocks that depend on it. The big slot's
            # projections trickle in piecewise (q/k/v) so no single burst
            # stalls the Exp stream.
            if NB == 4:
                big = NB - 1
                proj_q(0)
                late_loads()
                proj_k(0)
                proj_v(0)
                for tq in range(NT):
                    attn_block(0, tq)
                proj_phase(1)
                proj_q(big)
                for tq in range(NT):
                    attn_block(1, tq)
                proj_phase(2)
                proj_k(big)
                for tq in range(NT):
                    attn_block(2, tq)
                proj_v(big)
                norm_outproj_phase(0, last=False)
                attn_block(big, 0)
                norm_outproj_phase(1, last=False)
                attn_block(big, 1)
                norm_outproj_phase(2, last=False)
                attn_block(big, 2)
                attn_block(big, 3)
                norm_outproj_phase(big, last=True)
            else:
                late_loads()
                proj_phase(0)
                if NB > 1:
                    proj_phase(1)
                for s in range(NB):
                    for tq in range(NT):
                        attn_block(s, tq)
                    if s + 2 < NB:
                        proj_phase(s + 2)
                    if s >= 1:
                        norm_outproj_phase(s - 1, last=False)
                norm_outproj_phase(NB - 1, last=True)

    _split_multi_waits(nc)
    return nc


_CACHE = {}


def _get_nc(NB, J_list, NW_list):
    key = (NB, tuple(J_list), tuple(NW_list))
    if key not in _CACHE:
        _CACHE[key] = build_nc(NB, list(J_list), list(NW_list))
    return _CACHE[key]


def _xt(x, nw):
    """[T, D] -> [nw, P, KC, 512] transposed window-major layout."""
    xt = x.T.reshape(KC, P, NT, 512).transpose(2, 1, 0, 3)[:nw]
    return np.ascontiguousarray(xt).astype(BF16_NP)


def kernel(**inputs):
    query = np.asarray(inputs["query"], dtype=np.float32)
    key = np.asarray(inputs["key"], dtype=np.float32)
    value = np.asarray(inputs["value"], dtype=np.float32)
    vl = np.asarray(inputs["valid_length"]).astype(np.int64)
    W_q = np.asarray(inputs["W_q"], dtype=np.float32)
    b_q = np.asarray(inputs["b_q"], dtype=np.float32)
    W_k = np.asarray(inputs["W_k"], dtype=np.float32)
    b_k = np.asarray(inputs["b_k"], dtype=np.float32)
    W_v = np.asarray(inputs["W_v"], dtype=np.float32)
    b_v = np.asarray(inputs["b_v"], dtype=np.float32)
    W_o = np.asarray(inputs["W_o"], dtype=np.float32)
    b_o = np.asarray(inputs["b_o"], dtype=np.float32)

    B = query.shape[0]
    NB = B

    # slot s handles batch order[s] (ascending valid_len so the largest
    # attention phase runs last, hiding earlier slots' epilogues)
    order = np.argsort(vl, kind="stable")
    J_list, NW_list = [], []
    for s in range(NB):
        v = int(vl[order[s]])
        J = TC if v == 0 else max(1, -(-v // P))
        J_list.append(J)
        NW_list.append(-(-J // 4))

    nc = _get_nc(NB, J_list, NW_list)

    # host-side shard prep
    xq_np, xk_np, xv_np, mb_np = [], [], [], []
    for s in range(NB):
        b = int(order[s])
        v = int(vl[b])
        q_b = query[b] if v != 0 else np.zeros_like(query[b])
        xq_np.append(_xt(q_b, NT))
        xk_np.append(_xt(key[b], NW_list[s]))
        xv_np.append(_xt(value[b], NW_list[s]))
        rows = np.arange(P)[:, None] + P * np.arange(TC)[None, :]
        if v == 0:
            m = np.zeros((P, TC), np.float32)
        else:
            m = np.where(rows < v, 0.0, MASK_NEG).astype(np.float32)
        mb_np.append(m)

    # b_v folds into b_o: softmax rows sum to 1, so attn_out = P@(xWv) + b_v
    b_o_eff = b_o + b_v @ W_o

    i2_np = np.zeros((2, P), BF16_NP)
    i2_np[0, 0:DK] = 1
    i2_np[1, DK:P] = 1

    in_maps = []
    for c in range(NCORES):
        c0 = c * CPB
        cols = slice(c0, c0 + CPB)
        im = {
            "wq": np.ascontiguousarray(
                W_q.reshape(KC, P, H * DK).transpose(1, 0, 2)[:, :, cols]
            ).astype(BF16_NP),
            "wk": np.ascontiguousarray(
                W_k.reshape(KC, P, H * DK).transpose(1, 0, 2)[:, :, cols]
            ).astype(BF16_NP),
            "wv": np.ascontiguousarray(
                W_v.reshape(KC, P, H * DK).transpose(1, 0, 2)[:, :, cols]
            ).astype(BF16_NP),
            "wo": np.ascontiguousarray(W_o[cols]).astype(BF16_NP),
            "bq": b_q[cols].reshape(P, 1).astype(np.float32),
            "bk": b_k[cols].reshape(P, 1).astype(np.float32),
            "i2": i2_np,
        }
        for s in range(NB):
            im[f"xq{s}"] = xq_np[s]
            im[f"xk{s}"] = xk_np[s]
            im[f"xv{s}"] = xv_np[s]
            im[f"mb{s}"] = mb_np[s]
        in_maps.append(im)

    res = run_bass_kernel_spmd(nc, in_maps, list(range(NCORES)))

    out = np.zeros((B, T, D), np.float32)
    for s in range(NB):
        b = int(order[s])
        acc = np.zeros((T, D), np.float32)
        for c in range(NCORES):
            acc += np.asarray(res.results[c][f"o{s}"]).astype(np.float32)
        out[b] = acc + b_o_eff[None, :]
    return out


# revision 47
# speedup vs baseline: 1.6102x; 1.0247x over previous
"""Trainium2 Bass kernel for nn_MultiHeadAttention (B=4, T=2048, D=1024,
H=16, d_k=64) on 8 NeuronCores.

Sharding: tensor-parallel over heads - core c computes heads {2c, 2c+1} for
ALL batches (W_q/W_k/W_v column-sharded, W_o row-sharded). The final
all-reduce of the output projection is replaced by a host-side sum of the 8
partial outputs. Per-batch attention length (ceil(valid_len/128) Tk tiles)
is baked into the single SPMD program; every core owns 2 heads of every
batch so the instruction stream is identical and load-balanced.

Device tricks (v2):
  - scores^T layout (Tk on partitions, Tq on free). The two heads' QK^T
    matmuls write one [128, 2, 512] PSUM tile (adjacent row-tiled K=64
    matmuls -> concurrent on the PE array), and ONE Exp activation
    normalizes instruction overhead over both heads; the padding mask rides
    the per-partition bias operand.
  - softmax denominator: a ones-column folded into V (lhsT = [V_h | 1]).
  - b_v is folded into b_o on the host (softmax rows sum to 1).
  - Unnormalized [65, 512] outputs are staged to SBUF by DVE; den rows are
    gathered by SBUF-SBUF DMA so the reciprocal runs batched [8, 512];
    1/den is broadcast for BOTH heads with a single K=2 bf16 matmul
    (selector lhsT I2), then DVE multiplies into AO.
  - V projection accumulates 4 Tk tiles in one PSUM bank -> one [128, 512]
    stage-out copy per 512-row window.
  - K/V projections and DMAs only cover ceil(valid_len/512) windows.
  - trn2 encodes at most one semaphore wait per instruction; a post-pass
    splits multi-wait instructions Tile emits into single-wait events.
"""
import os
import sys

for _p in ("/opt/trn_rl_repo", "/root/.axon_site/_ro/trn_rl_repo"):
    if os.path.isdir(_p) and _p not in sys.path:
        sys.path.append(_p)

import numpy as np
import ml_dtypes

import concourse.bass as bass
import concourse.mybir as mybir
import concourse.tile as tile
from concourse.bass import ts
from concourse.bass_utils import run_bass_kernel_spmd

D = 1024
T = 2048
H = 16
DK = 64
P = 128
KC = D // P          # 8 contraction chunks for the projections
NT = T // 512        # 4 Tq chunks of 512
TC = T // P          # 16 Tk tiles / T chunks
NCORES = 8
CPB = (H // NCORES) * DK   # projection cols per core = 128 (2 heads)
MASK_NEG = -30000.0
EXP_SCALE = 0.125

F32 = mybir.dt.float32
BF16 = mybir.dt.bfloat16
AF = mybir.ActivationFunctionType
BF16_NP = ml_dtypes.bfloat16


def _split_multi_waits(nc):
    """trn2 instructions encode at most one sync wait; split the rest into
    standalone single-wait event-semaphore ops."""
    n_split = 0
    for f in nc.m.functions:
        for blk in f.blocks:
            insts = blk.instructions
            out = []
            changed = False
            for inst in insts:
                si = inst.sync_info
                if si is not None and len(si.on_wait) > 1:
                    waits = list(si.on_wait)
                    for k, wt in enumerate(waits[:-1]):
                        ev = mybir.InstEventSemaphore(
                            name=f"{inst.name}_wsplit{k}",
                            engine=inst.engine,
                            ins=[],
                            outs=[],
                            bass_nofuse=True,
                            sync_info=mybir.SyncInfo(on_wait=[wt], on_update=[]),
                        )
                        out.append(ev)
                        n_split += 1
                    inst.sync_info = mybir.SyncInfo(
                        on_wait=[waits[-1]], on_update=si.on_update
                    )
                    changed = True
                out.append(inst)
            if changed:
                blk.instructions = out
    return n_split


def build_nc(NB, J_list, NW_list):
    """Build the SPMD program.

    NB      : number of batch slots handled per core
    J_list  : per batch slot, number of 128-row Tk tiles of attention
    NW_list : per batch slot, number of 512-row K/V windows (= ceil(J/4))
    """
    nc = bass.Bass()

    xq_d = [nc.declare_dram_parameter(f"xq{s}", [NT, P, KC, 512], BF16,
                                      isOutput=False) for s in range(NB)]
    xk_d = [nc.declare_dram_parameter(f"xk{s}", [NW_list[s], P, KC, 512], BF16,
                                      isOutput=False) for s in range(NB)]
    xv_d = [nc.declare_dram_parameter(f"xv{s}", [NW_list[s], P, KC, 512], BF16,
                                      isOutput=False) for s in range(NB)]
    wq_d = nc.declare_dram_parameter("wq", [P, KC, CPB], BF16, isOutput=False)
    wk_d = nc.declare_dram_parameter("wk", [P, KC, CPB], BF16, isOutput=False)
    wv_d = nc.declare_dram_parameter("wv", [P, KC, CPB], BF16, isOutput=False)
    wo_d = nc.declare_dram_parameter("wo", [P, D], BF16, isOutput=False)
    bq_d = nc.declare_dram_parameter("bq", [P, 1], F32, isOutput=False)
    bk_d = nc.declare_dram_parameter("bk", [P, 1], F32, isOutput=False)
    mb_d = [nc.declare_dram_parameter(f"mb{s}", [P, TC], F32, isOutput=False)
            for s in range(NB)]
    i2_d = nc.declare_dram_parameter("i2", [2, P], BF16, isOutput=False)
    o_d = [nc.declare_dram_parameter(f"o{s}", [T, D], BF16, isOutput=True)
           for s in range(NB)]

    with tile.TileContext(nc) as tc:
        with (
            tc.tile_pool(name="pers", bufs=1) as pers,
            tc.tile_pool(name="stream", bufs=3) as stream,
            tc.tile_pool(name="attn", bufs=4) as attn_pool,
            tc.tile_pool(name="small", bufs=3) as small,
            tc.tile_pool(name="outp", bufs=4) as outp,
            tc.tile_pool(name="ps_qk", bufs=2, space="PSUM") as ps_qk,
            tc.tile_pool(name="ps_pv", bufs=2, space="PSUM") as ps_pv,
            tc.tile_pool(name="ps_gen", bufs=2, space="PSUM") as ps_gen,
        ):
            # ---- persistent tensors -------------------------------------
            # The Sync engine executes dma_start kicks IN ORDER; only
            # dependency-free prefetch loads may go through it. DMAs whose
            # source awaits compute (den gathers, rst2, o stores) go through
            # the otherwise-idle GpSimd queue so they never stall prefetch.
            wq = pers.tile([P, KC, CPB], BF16, name="wq")
            wk = pers.tile([P, KC, CPB], BF16, name="wk")
            wv = pers.tile([P, KC, CPB], BF16, name="wv")
            wo = pers.tile([P, D], BF16, name="wo")
            bq = pers.tile([P, 1], F32, name="bq")
            bk = pers.tile([P, 1], F32, name="bk")
            nc.sync.dma_start(wq[:], wq_d[:])

            def late_loads():
                nc.sync.dma_start(wk[:], wk_d[:])
                nc.sync.dma_start(wv[:], wv_d[:])
                nc.sync.dma_start(wo[:], wo_d[:])
                nc.sync.dma_start(bq[:], bq_d[:])
                nc.sync.dma_start(bk[:], bk_d[:])
                for s in range(NB):
                    nc.sync.dma_start(mb[s][:], mb_d[s][:])
                nc.sync.dma_start(I2[:], i2_d[:])
                # pull the one-time Exp table load off the first attention
                nc.vector.memset(warm[:], 0.0)
                nc.scalar.activation(warm[:], warm[:], AF.Exp)

            def pe_warm(n):
                # Junk LDWEIGHTS: keeps the PE array active through DMA
                # waits / engine-paced stretches so the HAM clock gate stays
                # at 8/8. Harmless - every real matmul reloads its weights.
                for _ in range(n):
                    nc.tensor.ldweights(wq[:, 0, :])

            mb = [pers.tile([P, TC], F32, name=f"mb{s}") for s in range(NB)]
            I2 = pers.tile([2, P], BF16, name="I2")
            warm = pers.tile([P, 1], F32, name="warm")

            QT = [pers.tile([P, T], BF16, name=f"QT{s}") for s in range(NB)]
            KT = [pers.tile([P, T], BF16, name=f"KT{s}") for s in range(NB)]
            # V with a ones column folded in at free index 64 of each head
            V = [pers.tile([P, TC, 2, DK + 1], BF16, name=f"V{s}")
                 for s in range(NB)]
            for s in range(NB):
                nc.vector.memset(V[s][:, :, :, DK], 1.0)

            AO = [pers.tile([P, T], BF16, name=f"AO{s}") for s in range(NB)]
            NR = 2 * NT  # (tq, head) rows per slot
            uo = [pers.tile([DK + 1, NR, 512], BF16, name=f"uo{s}")
                  for s in range(NB)]
            dens = [pers.tile([NR, 512], BF16, name=f"dens{s}")
                    for s in range(NB)]
            recs = [pers.tile([NR, 512], BF16, name=f"rec{s}")
                    for s in range(NB)]

            def proj_q(s):
                for n in range(NT):
                    xq_w = stream.tile([P, KC, 512], BF16, tag="xq_w")
                    nc.sync.dma_start(xq_w[:], xq_d[s][n])
                    ps_q = ps_gen.tile([P, 512], F32, tag="gen")
                    for kc in range(KC):
                        nc.tensor.matmul(ps_q[:], wq[:, kc, :],
                                         xq_w[:, kc, :],
                                         start=(kc == 0), stop=(kc == KC - 1))
                    nc.vector.tensor_scalar_add(QT[s][:, ts(n, 512)],
                                                ps_q[:], bq[:, 0:1])

            def proj_k(s):
                for n in range(NW_list[s]):
                    xk_w = stream.tile([P, KC, 512], BF16, tag="xk_w")
                    nc.sync.dma_start(xk_w[:], xk_d[s][n])
                    ps_k = ps_gen.tile([P, 512], F32, tag="gen")
                    for kc in range(KC):
                        nc.tensor.matmul(ps_k[:], wk[:, kc, :],
                                         xk_w[:, kc, :],
                                         start=(kc == 0), stop=(kc == KC - 1))
                    nc.vector.tensor_scalar_add(KT[s][:, ts(n, 512)],
                                                ps_k[:], bk[:, 0:1])

            def proj_v(s):
                # V projection: 4 Tk tiles share one PSUM bank per window
                for n in range(NW_list[s]):
                    xv_w = stream.tile([P, KC, 512], BF16, tag="xv_w")
                    nc.sync.dma_start(xv_w[:], xv_d[s][n])
                    ps_v = ps_gen.tile([P, 512], F32, tag="gen")
                    ps_v4 = ps_v[:].rearrange("p (t c) -> p t c", t=4)
                    for tc_i in range(4):
                        for kc in range(KC):
                            nc.tensor.matmul(ps_v4[:, tc_i, :],
                                             xv_w[:, kc, ts(tc_i, P)],
                                             wv[:, kc, :],
                                             start=(kc == 0),
                                             stop=(kc == KC - 1))
                    nc.vector.tensor_copy(
                        V[s][:, 4 * n:4 * n + 4, :, 0:DK],
                        ps_v4.rearrange("p t (h d) -> p t h d", d=DK))

            def proj_phase(s):
                proj_q(s)
                proj_k(s)
                proj_v(s)

            def attn_block(s, tq, warm_attn=False):
                J = J_list[s]
                if True:
                    po = [ps_pv.tile([DK + 1, 512], F32, tag="pv",
                                     name=f"po{h}") for h in range(2)]
                    for j in range(J):
                        ps_s = ps_qk.tile([P, 2, 512], F32, tag="qk")
                        nc.tensor.matmul(ps_s[:, 0, :],
                                         KT[s][0:DK, ts(j, P)],
                                         QT[s][0:DK, ts(tq, 512)],
                                         start=True, stop=True,
                                         tile_position=(0, 0))
                        nc.tensor.matmul(ps_s[:, 1, :],
                                         KT[s][DK:P, ts(j, P)],
                                         QT[s][DK:P, ts(tq, 512)],
                                         start=True, stop=True,
                                         tile_position=(DK, 0))
                        at = attn_pool.tile([P, 2, 512], BF16, tag="at")
                        nc.scalar.activation(at[:], ps_s[:], AF.Exp,
                                             bias=mb[s][:, j:j + 1],
                                             scale=EXP_SCALE)
                        for h01 in range(2):
                            nc.tensor.matmul(po[h01][:],
                                             V[s][:, j, h01, :],
                                             at[:, h01, :],
                                             start=(j == 0),
                                             stop=(j == J - 1))
                        if warm_attn:
                            pe_warm(2)
                    for h01 in range(2):
                        r = 2 * tq + h01
                        nc.vector.tensor_copy(uo[s][:, r, :], po[h01][:])
                        nc.sync.dma_start(dens[s][r:r + 1, :],
                                          uo[s][DK:DK + 1, r, :])

            def norm_outproj_phase(s, last):
                if last:
                    pe_warm(24)
                with nc.allow_low_precision(reason="bf16 softmax denom"):
                    nc.vector.reciprocal(recs[s][:], dens[s][:])
                for tq in range(NT):
                    rst2 = small.tile([2, 512], BF16, tag="rst")
                    nc.sync.dma_start(rst2[:], recs[s][2 * tq:2 * tq + 2, :])
                    ps_b = ps_gen.tile([P, 512], F32, tag="gen")
                    nc.tensor.matmul(ps_b[:], I2[:], rst2[:],
                                     start=True, stop=True)
                    if last:
                        pe_warm(8)
                    for h01 in range(2):
                        nc.vector.tensor_mul(
                            out=AO[s][ts(h01, DK), ts(tq, 512)],
                            in0=ps_b[ts(h01, DK), :],
                            in1=uo[s][0:DK, 2 * tq + h01, :])

                # ---- output projection (partial; host sums cores) -------
                for m in range(TC):
                    ot = outp.tile([P, D], BF16, tag="ot")
                    for n2 in range(2):
                        ps_op = ps_gen.tile([P, 512], F32, tag="gen")
                        nc.tensor.matmul(ps_op[:], AO[s][:, ts(m, P)],
                                         wo[:, ts(n2, 512)],
                                         start=True, stop=True)
                        if last:
                            pe_warm(2)
                        if (2 * m + n2) % 3 == (1 if last else 0):
                            nc.scalar.activation(ot[:, ts(n2, 512)], ps_op[:],
                                                 AF.Identity)
                        else:
                            nc.vector.tensor_copy(ot[:, ts(n2, 512)], ps_op[:])
                    nc.sync.dma_start(o_d[s][ts(m, P), :], ot[:])

            # Software-pipelined emission. Slots are ordered ascending by J.
            # Strict-priority scheduling means work emitted EARLIER preempts
            # later work, so each proj/norm sub-phase is emitted right AFTER
            # the attention blocks whose ACT-paced stretch should hide it,
            # and right BEFORE the blocks that depend on it. The big slot's
            # projections trickle in piecewise (q/k/v) so no single burst
            # stalls the Exp stream.
            if NB == 4:
                big = NB - 1
                pe_warm(40)
                proj_q(0)
                late_loads()
                proj_k(0)
                proj_v(0)
                for tq in range(NT):
                    attn_block(0, tq)
                proj_phase(1)
                proj_q(big)
                for tq in range(NT):
                    attn_block(1, tq)
                proj_phase(2)
                proj_k(big)
                for tq in range(NT):
                    attn_block(2, tq)
                proj_v(big)
                norm_outproj_phase(0, last=False)
                attn_block(big, 0)
                norm_outproj_phase(1, last=False)
                attn_block(big, 1)
                norm_outproj_phase(2, last=False)
                attn_block(big, 2, warm_attn=True)
                attn_block(big, 3, warm_attn=True)
                norm_outproj_phase(big, last=True)
            else:
                late_loads()
                proj_phase(0)
                if NB > 1:
                    proj_phase(1)
                for s in range(NB):
                    for tq in range(NT):
                        attn_block(s, tq)
                    if s + 2 < NB:
                        proj_phase(s + 2)
                    if s >= 1:
                        norm_outproj_phase(s - 1, last=False)
                norm_outproj_phase(NB - 1, last=True)

    _split_multi_waits(nc)
    return nc


_CACHE = {}


def _get_nc(NB, J_list, NW_list):
    key = (NB, tuple(J_list), tuple(NW_list))
    if key not in _CACHE:
        _CACHE[key] = build_nc(NB, list(J_list), list(NW_list))
    return _CACHE[key]


def _xt(x, nw):
    """[T, D] -> [nw, P, KC, 512] transposed window-major layout."""
    xt = x.T.reshape(KC, P, NT, 512).transpose(2, 1, 0, 3)[:nw]
    return np.ascontiguousarray(xt).astype(BF16_NP)


def kernel(**inputs):
    query = np.asarray(inputs["query"], dtype=np.float32)
    key = np.asarray(inputs["key"], dtype=np.float32)
    value = np.asarray(inputs["value"], dtype=np.float32)
    vl = np.asarray(inputs["valid_length"]).astype(np.int64)
    W_q = np.asarray(inputs["W_q"], dtype=np.float32)
    b_q = np.asarray(inputs["b_q"], dtype=np.float32)
    W_k = np.asarray(inputs["W_k"], dtype=np.float32)
    b_k = np.asarray(inputs["b_k"], dtype=np.float32)
    W_v = np.asarray(inputs["W_v"], dtype=np.float32)
    b_v = np.asarray(inputs["b_v"], dtype=np.float32)
    W_o = np.asarray(inputs["W_o"], dtype=np.float32)
    b_o = np.asarray(inputs["b_o"], dtype=np.float32)

    B = query.shape[0]
    NB = B

    # slot s handles batch order[s] (ascending valid_len so the largest
    # attention phase runs last, hiding earlier slots' epilogues)
    order = np.argsort(vl, kind="stable")
    J_list, NW_list = [], []
    for s in range(NB):
        v = int(vl[order[s]])
        J = TC if v == 0 else max(1, -(-v // P))
        J_list.append(J)
        NW_list.append(-(-J // 4))

    nc = _get_nc(NB, J_list, NW_list)

    # host-side shard prep
    xq_np, xk_np, xv_np, mb_np = [], [], [], []
    for s in range(NB):
        b = int(order[s])
        v = int(vl[b])
        q_b = query[b] if v != 0 else np.zeros_like(query[b])
        xq_np.append(_xt(q_b, NT))
        xk_np.append(_xt(key[b], NW_list[s]))
        xv_np.append(_xt(value[b], NW_list[s]))
        rows = np.arange(P)[:, None] + P * np.arange(TC)[None, :]
        if v == 0:
            m = np.zeros((P, TC), np.float32)
        else:
            m = np.where(rows < v, 0.0, MASK_NEG).astype(np.float32)
        mb_np.append(m)

    # b_v folds into b_o: softmax rows sum to 1, so attn_out = P@(xWv) + b_v
    b_o_eff = b_o + b_v @ W_o

    i2_np = np.zeros((2, P), BF16_NP)
    i2_np[0, 0:DK] = 1
    i2_np[1, DK:P] = 1

    in_maps = []
    for c in range(NCORES):
        c0 = c * CPB
        cols = slice(c0, c0 + CPB)
        im = {
            "wq": np.ascontiguousarray(
                W_q.reshape(KC, P, H * DK).transpose(1, 0, 2)[:, :, cols]
            ).astype(BF16_NP),
            "wk": np.ascontiguousarray(
                W_k.reshape(KC, P, H * DK).transpose(1, 0, 2)[:, :, cols]
            ).astype(BF16_NP),
            "wv": np.ascontiguousarray(
                W_v.reshape(KC, P, H * DK).transpose(1, 0, 2)[:, :, cols]
            ).astype(BF16_NP),
            "wo": np.ascontiguousarray(W_o[cols]).astype(BF16_NP),
            "bq": b_q[cols].reshape(P, 1).astype(np.float32),
            "bk": b_k[cols].reshape(P, 1).astype(np.float32),
            "i2": i2_np,
        }
        for s in range(NB):
            im[f"xq{s}"] = xq_np[s]
            im[f"xk{s}"] = xk_np[s]
            im[f"xv{s}"] = xv_np[s]
            im[f"mb{s}"] = mb_np[s]
        in_maps.append(im)

    def run_once():
        res = run_bass_kernel_spmd(nc, in_maps, list(range(NCORES)))
        out = np.zeros((B, T, D), np.float32)
        for s in range(NB):
            b = int(order[s])
            acc = np.zeros((T, D), np.float32)
            for c in range(NCORES):
                acc += np.asarray(res.results[c][f"o{s}"]).astype(np.float32)
            out[b] = acc + b_o_eff[None, :]
        return out

    # A freshly-reset device occasionally corrupts its first execution; run
    # until two consecutive runs agree (normally exactly 2 runs, 1 compile).
    prev = run_once()
    for _ in range(3):
        cur = run_once()
        if (np.isfinite(prev).all() and np.isfinite(cur).all()
                and float(np.abs(cur - prev).max()) < 1e-4):
            return cur
        prev = cur
    return prev
